# revision 31
# baseline (speedup 1.0000x reference)
"""MultiHeadLatentAttention TRN2 kernel.

Sharding: 8 cores = 2 batches x 4 head-groups (4 heads of 128 dims each).
Each core computes, for its (batch, 4 heads):
    qT_h = Wq_h^T xT          [hd, S]     (fp16 matmuls, fp32 psum)
    latT = Wdkv^T xT          [256, S]
    kT_h = Wuk_h^T latT       [hd, S]
    v_h  = latT^T Wuv_h       [S, hd]
    scoresT = k qT            [keys, q]   (transposed scores: no transposes needed)
    expT = exp(scale*scoresT) (causal: skip invalid blocks, tri-mask diagonal)
    den  = ones^T expT        [128, q]    (all-ones stationary matmul = sum over
                                           keys AND broadcast across partitions)
    ctxT = v^T expT / den     [hd, q]
    part = sum_h ctxT_h^T Wout_h  [S, dout]  (row-parallel out-proj partial)
Host sums the 4 partials per batch and adds b_out.
"""

import sys

_BASS_REPO = "/opt/trn_rl_repo"
if _BASS_REPO not in sys.path:
    sys.path.insert(0, _BASS_REPO)

import numpy as np

import concourse.bass as bass  # noqa: F401
import concourse.mybir as mybir
import concourse.tile as tile
from concourse import bacc, bass_utils

F32 = mybir.dt.float32
F16 = mybir.dt.float16

B = 2
S = 2048
DIN = 2048
DOUT = 2048
NH = 16
HD = 128
LAT = 256
NCORES = 8
HEADS_PER_CORE = 4
COLS_PER_CORE = HEADS_PER_CORE * HD  # 512

KC = DIN // 128  # 16 contraction chunks over d_in
NB = S // 512    # 4 blocks of 512 over S
NT = S // 128    # 16 tiles of 128 over S
SCALE = 1.0 / float(np.sqrt(HD))

_CACHE = {}


def _build():
    nc = bacc.Bacc("TRN2", target_bir_lowering=False, debug=False,
                   num_devices=NCORES)

    xt_d = nc.dram_tensor("xt", [DIN, S], F16, kind="ExternalInput")
    wq_d = nc.dram_tensor("wq", [DIN, COLS_PER_CORE], F16, kind="ExternalInput")
    wdkv_d = nc.dram_tensor("wdkv", [DIN, LAT], F16, kind="ExternalInput")
    wuk_d = nc.dram_tensor("wuk", [LAT, COLS_PER_CORE], F16, kind="ExternalInput")
    wuv_d = nc.dram_tensor("wuv", [LAT, COLS_PER_CORE], F16, kind="ExternalInput")
    wout_d = nc.dram_tensor("wout", [COLS_PER_CORE, DOUT], F16, kind="ExternalInput")
    mask_d = nc.dram_tensor("mask", [128, 128], F16, kind="ExternalInput")
    out_d = nc.dram_tensor("out", [S, DOUT], F16, kind="ExternalOutput")

    Exp = mybir.ActivationFunctionType.Exp

    with tile.TileContext(nc) as tc:
        with (
            tc.tile_pool(name="consts", bufs=1) as cpool,
            tc.tile_pool(name="wts", bufs=1) as wpool,
            tc.tile_pool(name="acts", bufs=1) as apool,
            tc.tile_pool(name="temps", bufs=1) as tpool,
        ):
            # ---- constants ----
            ones_t = cpool.tile([128, 512], F16, name="ones_t", tag="ones_t")
            nc.vector.memset(ones_t[:], 1.0)
            neg_t = cpool.tile([128, 128], F16, name="neg_t", tag="neg_t")
            nc.vector.memset(neg_t[:], -30000.0)
            mask_t = cpool.tile([128, 128], F16, name="mask_t", tag="mask_t")
            nc.scalar.dma_start(mask_t[:], mask_d.ap())

            # ---- weights ----
            # Spread input DMAs over 4 engine queues (sync/gpsimd/vector/
            # scalar) so no single queue's trigger cost or head-of-line
            # blocking stalls the first matmuls. Per queue: its wdkv chunks
            # first (tiny, needed first), then its xt chunks, then wq, then
            # the later-phase weights.
            qengs = [nc.sync, nc.gpsimd, nc.scalar]
            rr = [0]

            def ld(tile_ap, dram_ap):
                qengs[rr[0] % 3].dma_start(tile_ap, dram_ap)
                rr[0] += 1

            # Issue loads in CONSUMPTION order (sweeps are chunk-major), with
            # each xt chunk split over the three queues so chunk k lands
            # with minimal skew (arrival order == consumption order).
            xt = []
            wdkv = []
            for k in range(KC):
                # per chunk: xt split in two 2KB-line halves on two queues,
                # wdkv on the third; roles rotate so the queues stay balanced
                # and chunk k's inputs arrive together (sweep 1 consumes at
                # ~1.7us/chunk, matching the ~1.75us/chunk aggregate stream)
                xtile = wpool.tile([128, S], F16, name=f"xt{k}", tag=f"xt{k}")
                dtile = wpool.tile([128, LAT], F16, name=f"wdkv{k}",
                                   tag=f"wdkv{k}")
                qA, qB, qC = (qengs[(k + i) % 3] for i in range(3))
                qA.dma_start(xtile[:, 0:S // 2],
                             xt_d.ap()[128 * k:128 * (k + 1), 0:S // 2])
                qB.dma_start(xtile[:, S // 2:S],
                             xt_d.ap()[128 * k:128 * (k + 1), S // 2:S])
                qC.dma_start(dtile[:], wdkv_d.ap()[128 * k:128 * (k + 1), :])
                xt.append(xtile)
                wdkv.append(dtile)
            wq = []
            for k in range(KC):
                t = wpool.tile([128, COLS_PER_CORE], F16, name=f"wq{k}",
                               tag=f"wq{k}")
                ld(t[:], wq_d.ap()[128 * k:128 * (k + 1), :])
                wq.append(t)
            wuk = []
            wuv = []
            for m in range(LAT // 128):
                t = wpool.tile([128, COLS_PER_CORE], F16, name=f"wuk{m}", tag=f"wuk{m}")
                ld(t[:], wuk_d.ap()[128 * m:128 * (m + 1), :])
                wuk.append(t)
                t = wpool.tile([128, COLS_PER_CORE], F16, name=f"wuv{m}", tag=f"wuv{m}")
                ld(t[:], wuv_d.ap()[128 * m:128 * (m + 1), :])
                wuv.append(t)
            wout = []
            for h in range(HEADS_PER_CORE):
                t = wpool.tile([128, DOUT], F16, name=f"wout{h}", tag=f"wout{h}")
                ld(t[:], wout_d.ap()[128 * h:128 * (h + 1), :])
                wout.append(t)

            # ---- persistent activations ----
            latT = [apool.tile([128, S], F16, name=f"latT{m}", tag=f"latT{m}")
                    for m in range(LAT // 128)]
            qT = [apool.tile([128, S], F16, name=f"qT{h}", tag=f"qT{h}")
                  for h in range(HEADS_PER_CORE)]
            kT = [apool.tile([128, S], F16, name=f"kT{h}", tag=f"kT{h}")
                  for h in range(HEADS_PER_CORE)]
            # v stored 4-heads-wide: col = 512*stt + 128*h + d
            vt_all = apool.tile([128, 4 * S], F16, name="vt_all", tag="vt_all")
            ctxT = [apool.tile([128, S], F16, name=f"ctxT{h}", tag=f"ctxT{h}")
                    for h in range(HEADS_PER_CORE)]

            # ================= phase 1: projections =================
            with tc.tile_pool(name="pproj", bufs=8, space="PSUM") as pproj:
                # PE warmup: HAM-warm the array while input DMAs stream in.
                warm = pproj.tile([128, 512], F32, name="warm", tag="pp")
                for _ in range(40):
                    nc.tensor.matmul(warm[:, 0:128], ones_t[:, 0:128],
                                     ones_t[:, 0:128], start=True, stop=True)

                def kmajor(groups, lhs_of, rhs_of, nk, out_of, copy_eng="scalar"):
                    """Accumulate len(groups) psum banks over nk chunks,
                    chunk-major so compute starts on the first DMA."""
                    pls = [pproj.tile([128, 512], F32, name=f"pp{i}", tag="pp")
                           for i in range(len(groups))]
                    for k in range(nk):
                        for i, g in enumerate(groups):
                            nc.tensor.matmul(pls[i][:], lhs_of(k, g), rhs_of(k, g),
                                             start=(k == 0), stop=(k == nk - 1))
                    for i, g in enumerate(groups):
                        if copy_eng == "scalar":
                            nc.scalar.copy(out_of(g), pls[i][:])
                        else:
                            nc.vector.tensor_copy(out_of(g), pls[i][:])

                # sweep 1: latT = Wdkv^T xT (8 groups: 2 m x 4 sb),
                # chunk-major so compute tracks the xt stream
                kmajor(
                    [(m, sb) for m in range(2) for sb in range(NB)],
                    lambda k, g: wdkv[k][:, 128 * g[0]:128 * (g[0] + 1)],
                    lambda k, g: xt[k][:, 512 * g[1]:512 * (g[1] + 1)],
                    KC,
                    lambda g: latT[g[0]][:, 512 * g[1]:512 * (g[1] + 1)])

                # sweeps 2-3: qT (two batches of 8 groups: 2 h x 4 sb)
                for h0 in (0, 2):
                    kmajor(
                        [(h0 + dh, sb) for dh in range(2) for sb in range(NB)],
                        lambda k, g: wq[k][:, 128 * g[0]:128 * (g[0] + 1)],
                        lambda k, g: xt[k][:, 512 * g[1]:512 * (g[1] + 1)],
                        KC,
                        lambda g: qT[g[0]][:, 512 * g[1]:512 * (g[1] + 1)])

                # kT_h = Wuk_h^T latT
                kmajor(
                    [(h, sb) for h in range(2) for sb in range(NB)],
                    lambda k, g: wuk[k][:, 128 * g[0]:128 * (g[0] + 1)],
                    lambda k, g: latT[k][:, 512 * g[1]:512 * (g[1] + 1)],
                    2,
                    lambda g: kT[g[0]][:, 512 * g[1]:512 * (g[1] + 1)],
                    copy_eng="vector")
                kmajor(
                    [(h, sb) for h in (2, 3) for sb in range(NB)],
                    lambda k, g: wuk[k][:, 128 * g[0]:128 * (g[0] + 1)],
                    lambda k, g: latT[k][:, 512 * g[1]:512 * (g[1] + 1)],
                    2,
                    lambda g: kT[g[0]][:, 512 * g[1]:512 * (g[1] + 1)],
                    copy_eng="vector")

                # v = latT^T Wuv, all 4 heads per matmul (512-wide moving
                # operand keeps LDWEIGHTS hidden); out tile = [s(128), 4h*hd]
                for stt in range(NT):
                    pv = pproj.tile([128, 512], F32, name="pv", tag="pp")
                    for m in range(LAT // 128):
                        nc.tensor.matmul(
                            pv[:],
                            latT[m][:, 128 * stt:128 * (stt + 1)],
                            wuv[m][:],
                            start=(m == 0), stop=(m == LAT // 128 - 1))
                    # alternate copy engines so the psum banks drain fast
                    # enough for the first attention scores
                    if stt % 2 == 0:
                        nc.vector.tensor_copy(
                            vt_all[:, 512 * stt:512 * (stt + 1)], pv[:])
                    else:
                        nc.scalar.copy(
                            vt_all[:, 512 * stt:512 * (stt + 1)], pv[:])

            # ========= phase 2: attention + interleaved out-proj =========
            # key tiles processed in pairs -> one [128,1024] exp per pair.
            # qb-outer / h-inner so each q-block's out-projection (PE-heavy,
            # ACT-idle) overlaps the next block's ACT-paced attention.
            with (
                tc.tile_pool(name="psc", bufs=2, space="PSUM") as psc,
                tc.tile_pool(name="pctx", bufs=2, space="PSUM") as pctx,
                tc.tile_pool(name="pden", bufs=2, space="PSUM") as pden,
            ):
                for qb in range(NB):
                    for h in range(HEADS_PER_CORE):
                        ps_ctx = pctx.tile([128, 512], F32, name="ps_ctx", tag="ctx")
                        ps_den = pden.tile([128, 512], F32, name="ps_den", tag="den")
                        nkt = 4 * qb + 4
                        exs_hold = None
                        den_started = False
                        for kt0 in range(0, nkt, 2):
                            pair = (kt0, kt0 + 1)
                            # valid q start (block-local) per kt; pair shares
                            # the wider (earlier) start col0 of ktA
                            djA = pair[0] - 4 * qb
                            col0 = 128 * djA if djA >= 0 else 0
                            qhi = 512 * (qb + 1)
                            ps_sc = psc.tile([128, 1024], F32, name="ps_sc",
                                             tag="sc")
                            ex = tpool.tile([128, 1024], F16, name="ex", tag="ex",
                                            bufs=3)
                            for half, kt in enumerate(pair):
                                dj = kt - 4 * qb
                                c = 128 * dj if dj >= 0 else 0
                                # fill with -huge ONLY the strip that both
                                # lies in the exp window [col0:1024] and is
                                # read by the pair-add/ctx consumers
                                # ([col0:512] and [512+col0:1024]); the rest
                                # of the invalid region is either below col0
                                # (never exp'd) or exp's garbage that no
                                # consumer reads.
                                flo = max(512 * half + col0, col0)
                                fhi = 512 * half + c
                                if fhi > flo:
                                    nc.tensor.matmul(
                                        ps_sc[:, flo:fhi],
                                        neg_t[:], ones_t[:, 0:fhi - flo],
                                        start=True, stop=True,
                                        skip_group_check=True)
                                nc.tensor.matmul(
                                    ps_sc[:, 512 * half + c:512 * (half + 1)],
                                    kT[h][:, 128 * kt:128 * (kt + 1)],
                                    qT[h][:, 512 * qb + c:qhi],
                                    start=True, stop=True,
                                    skip_group_check=True)
                            # one wide exp for the pair (psum -> sbuf fp16)
                            nc.scalar.activation(ex[:, col0:1024],
                                                 ps_sc[:, col0:1024], Exp,
                                                 scale=SCALE)
                            for half, kt in enumerate(pair):
                                dj = kt - 4 * qb
                                if dj >= 0:
                                    c = 128 * dj
                                    nc.vector.tensor_mul(
                                        ex[:, 512 * half + c:512 * half + c + 128],
                                        ex[:, 512 * half + c:512 * half + c + 128],
                                        mask_t[:])
                            # pair-sum on DVE halves the denominator matmuls;
                            # off-diagonal quads get a second-level quad-sum
                            # so one den matmul covers 4 key tiles
                            exs = tpool.tile([128, 512], F16, name="exs",
                                             tag="exs", bufs=3)
                            nc.vector.tensor_add(exs[:, col0:512],
                                                 ex[:, col0:512],
                                                 ex[:, 512 + col0:1024])
                            if (kt0 // 4) < qb:
                                if kt0 % 4 == 0:
                                    exs_hold = exs
                                else:
                                    exq = tpool.tile([128, 512], F16, name="exq",
                                                     tag="exq", bufs=2)
                                    nc.vector.tensor_add(exq[:], exs_hold[:],
                                                         exs[:])
                                    nc.tensor.matmul(
                                        ps_den[:], ones_t[:, 0:128], exq[:],
                                        start=(not den_started), stop=False)
                                    den_started = True
                            else:
                                nc.tensor.matmul(
                                    ps_den[:, col0:512],
                                    ones_t[:, 0:128],
                                    exs[:, col0:512],
                                    start=(not den_started),
                                    stop=(kt0 == nkt - 2))
                                den_started = True
                            for half, kt in enumerate(pair):
                                nc.tensor.matmul(
                                    ps_ctx[:, col0:512],
                                    vt_all[:, 512 * kt + 128 * h:
                                           512 * kt + 128 * (h + 1)],
                                    ex[:, 512 * half + col0:512 * (half + 1)],
                                    start=(kt0 == 0 and half == 0),
                                    stop=(kt == nkt - 1))
                        rden = tpool.tile([128, 512], F32, name="rden", tag="rden",
                                          bufs=2)
                        nc.vector.reciprocal_approx_fast(rden[:], ps_den[:])
                        nc.vector.tensor_mul(ctxT[h][:, 512 * qb:512 * (qb + 1)],
                                             ps_ctx[:], rden[:])

                    # out-projection for this q-block's 4 S-tiles
                    # (psum slots shared with the den tag). Copies alternate
                    # vector/scalar so the po psum slot frees at 2x rate;
                    # output staged+stored as fp16 (halves store volume),
                    # store triggers on sync/gpsimd (both idle here).
                    for stt in range(4 * qb, 4 * qb + 4):
                        for ob in range(NB):
                            po = pden.tile([128, 512], F32, name="po", tag="den")
                            for h in range(HEADS_PER_CORE):
                                nc.tensor.matmul(
                                    po[:],
                                    ctxT[h][:, 128 * stt:128 * (stt + 1)],
                                    wout[h][:, 512 * ob:512 * (ob + 1)],
                                    start=(h == 0), stop=(h == HEADS_PER_CORE - 1))
                            osb = tpool.tile([128, 512], F16, name="osb", tag="osb",
                                             bufs=4)
                            if (stt + ob) % 2 == 0:
                                nc.scalar.copy(osb[:], po[:])
                                st_eng = nc.gpsimd
                            else:
                                nc.vector.tensor_copy(osb[:], po[:])
                                st_eng = nc.sync
                            st_eng.dma_start(
                                out_d.ap()[128 * stt:128 * (stt + 1),
                                           512 * ob:512 * (ob + 1)],
                                osb[:])

    nc.compile()
    return nc


def _get_nc():
    if "nc" not in _CACHE:
        _CACHE["nc"] = _build()
    return _CACHE["nc"]


def _make_in_maps(x, W_query, W_DKV, W_UK, W_UV, W_out):
    mask = np.triu(np.ones((128, 128), dtype=np.float16))
    wdkv16 = W_DKV.astype(np.float16)
    xT16 = [x[b].T.astype(np.float16) for b in range(B)]
    in_maps = []
    for c in range(NCORES):
        b = c // 4
        g = c % 4
        cols = slice(512 * g, 512 * (g + 1))
        in_maps.append({
            "xt": xT16[b],
            "wq": W_query[:, cols].astype(np.float16),
            "wdkv": wdkv16,
            "wuk": W_UK[:, cols].astype(np.float16),
            "wuv": W_UV[:, cols].astype(np.float16),
            "wout": W_out[cols, :].astype(np.float16),
            "mask": mask,
        })
    return in_maps


def run_on_device(x, W_query, W_DKV, W_UK, W_UV, W_out, **run_kwargs):
    nc = _get_nc()
    in_maps = _make_in_maps(x, W_query, W_DKV, W_UK, W_UV, W_out)
    return bass_utils.run_bass_kernel_spmd(
        nc, in_maps, core_ids=list(range(NCORES)), **run_kwargs)


def kernel(x, W_query, W_DKV, W_UK, W_UV, W_out, b_out):
    x = np.asarray(x, dtype=np.float32)
    W_query = np.asarray(W_query, dtype=np.float32)
    W_DKV = np.asarray(W_DKV, dtype=np.float32)
    W_UK = np.asarray(W_UK, dtype=np.float32)
    W_UV = np.asarray(W_UV, dtype=np.float32)
    W_out = np.asarray(W_out, dtype=np.float32)
    b_out = np.asarray(b_out, dtype=np.float32)

    res = None
    for attempt in range(3):
        try:
            res = run_on_device(x, W_query, W_DKV, W_UK, W_UV, W_out)
            break
        except Exception:
            if attempt == 2:
                raise
    out = np.empty((B, S, DOUT), dtype=np.float32)
    for b in range(B):
        acc = res.results[4 * b]["out"].astype(np.float32)
        for g in range(1, 4):
            acc += res.results[4 * b + g]["out"].astype(np.float32)
        out[b] = acc + b_out[None, :]
    return out



# revision 34
# speedup vs baseline: 1.0221x; 1.0221x over previous
"""MultiHeadLatentAttention TRN2 kernel.

Sharding: 8 cores = 2 batches x 4 head-groups (4 heads of 128 dims each).
Each core computes, for its (batch, 4 heads):
    qT_h = Wq_h^T xT          [hd, S]     (fp16 matmuls, fp32 psum)
    latT = Wdkv^T xT          [256, S]
    kT_h = Wuk_h^T latT       [hd, S]
    v_h  = latT^T Wuv_h       [S, hd]
    scoresT = k qT            [keys, q]   (transposed scores: no transposes needed)
    expT = exp(scale*scoresT) (causal: skip invalid blocks, tri-mask diagonal)
    den  = ones^T expT        [128, q]    (all-ones stationary matmul = sum over
                                           keys AND broadcast across partitions)
    ctxT = v^T expT / den     [hd, q]
    part = sum_h ctxT_h^T Wout_h  [S, dout]  (row-parallel out-proj partial)
Host sums the 4 partials per batch and adds b_out.
"""

import sys

_BASS_REPO = "/opt/trn_rl_repo"
if _BASS_REPO not in sys.path:
    sys.path.insert(0, _BASS_REPO)

import numpy as np

import concourse.bass as bass  # noqa: F401
import concourse.mybir as mybir
import concourse.tile as tile
from concourse import bacc, bass_utils

F32 = mybir.dt.float32
F16 = mybir.dt.float16

B = 2
S = 2048
DIN = 2048
DOUT = 2048
NH = 16
HD = 128
LAT = 256
NCORES = 8
HEADS_PER_CORE = 4
COLS_PER_CORE = HEADS_PER_CORE * HD  # 512

KC = DIN // 128  # 16 contraction chunks over d_in
NB = S // 512    # 4 blocks of 512 over S
NT = S // 128    # 16 tiles of 128 over S
SCALE = 1.0 / float(np.sqrt(HD))

_CACHE = {}


def _build():
    nc = bacc.Bacc("TRN2", target_bir_lowering=False, debug=False,
                   num_devices=NCORES)

    xt_d = nc.dram_tensor("xt", [DIN, S], F16, kind="ExternalInput")
    wq_d = nc.dram_tensor("wq", [DIN, COLS_PER_CORE], F16, kind="ExternalInput")
    # per-core latent half: cores with (core%4)//2==0 get W_DKV[:,0:128],
    # the others get W_DKV[:,128:256]; a pairwise AllGather reassembles
    # the full latent in rank order so the kernel stays SPMD-identical
    wdkv_d = nc.dram_tensor("wdkv", [DIN, LAT // 2], F16, kind="ExternalInput")
    wuk_d = nc.dram_tensor("wuk", [LAT, COLS_PER_CORE], F16, kind="ExternalInput")
    wuv_d = nc.dram_tensor("wuv", [LAT, COLS_PER_CORE], F16, kind="ExternalInput")
    wout_d = nc.dram_tensor("wout", [COLS_PER_CORE, DOUT], F16, kind="ExternalInput")
    mask_d = nc.dram_tensor("mask", [128, 128], F16, kind="ExternalInput")
    out_d = nc.dram_tensor("out", [S, DOUT], F16, kind="ExternalOutput")

    Exp = mybir.ActivationFunctionType.Exp

    with tile.TileContext(nc) as tc:
        with (
            tc.tile_pool(name="consts", bufs=1) as cpool,
            tc.tile_pool(name="wts", bufs=1) as wpool,
            tc.tile_pool(name="acts", bufs=1) as apool,
            tc.tile_pool(name="temps", bufs=1) as tpool,
        ):
            # ---- constants ----
            ones_t = cpool.tile([128, 512], F16, name="ones_t", tag="ones_t")
            nc.vector.memset(ones_t[:], 1.0)
            neg_t = cpool.tile([128, 128], F16, name="neg_t", tag="neg_t")
            nc.vector.memset(neg_t[:], -30000.0)
            mask_t = cpool.tile([128, 128], F16, name="mask_t", tag="mask_t")
            nc.scalar.dma_start(mask_t[:], mask_d.ap())

            # ---- weights ----
            # Spread input DMAs over 4 engine queues (sync/gpsimd/vector/
            # scalar) so no single queue's trigger cost or head-of-line
            # blocking stalls the first matmuls. Per queue: its wdkv chunks
            # first (tiny, needed first), then its xt chunks, then wq, then
            # the later-phase weights.
            qengs = [nc.sync, nc.gpsimd, nc.scalar]
            rr = [0]

            def ld(tile_ap, dram_ap):
                qengs[rr[0] % 3].dma_start(tile_ap, dram_ap)
                rr[0] += 1

            # Issue loads in CONSUMPTION order (sweeps are chunk-major), with
            # each xt chunk split over the three queues so chunk k lands
            # with minimal skew (arrival order == consumption order).
            wdkv = []
            for k in range(KC):
                t = wpool.tile([128, LAT // 2], F16, name=f"wdkv{k}",
                               tag=f"wdkv{k}")
                ld(t[:], wdkv_d.ap()[128 * k:128 * (k + 1), :])
                wdkv.append(t)
            xt = []
            wq = []
            for k in range(KC):
                # per chunk: xt split in two 2KB-line halves on two queues,
                # wq (full tile, 1KB lines) on the third; roles rotate so the
                # queues stay balanced and chunk k's inputs arrive together
                xtile = wpool.tile([128, S], F16, name=f"xt{k}", tag=f"xt{k}")
                qtile = wpool.tile([128, COLS_PER_CORE], F16, name=f"wq{k}",
                                   tag=f"wq{k}")
                qA, qB, qC = (qengs[(k + i) % 3] for i in range(3))
                qA.dma_start(xtile[:, 0:S // 2],
                             xt_d.ap()[128 * k:128 * (k + 1), 0:S // 2])
                qB.dma_start(xtile[:, S // 2:S],
                             xt_d.ap()[128 * k:128 * (k + 1), S // 2:S])
                qC.dma_start(qtile[:], wq_d.ap()[128 * k:128 * (k + 1), :])
                xt.append(xtile)
                wq.append(qtile)
            wuk = []
            wuv = []
            for m in range(LAT // 128):
                t = wpool.tile([128, COLS_PER_CORE], F16, name=f"wuk{m}", tag=f"wuk{m}")
                ld(t[:], wuk_d.ap()[128 * m:128 * (m + 1), :])
                wuk.append(t)
                t = wpool.tile([128, COLS_PER_CORE], F16, name=f"wuv{m}", tag=f"wuv{m}")
                ld(t[:], wuv_d.ap()[128 * m:128 * (m + 1), :])
                wuv.append(t)
            wout = []
            for h in range(HEADS_PER_CORE):
                t = wpool.tile([128, DOUT], F16, name=f"wout{h}", tag=f"wout{h}")
                ld(t[:], wout_d.ap()[128 * h:128 * (h + 1), :])
                wout.append(t)

            # ---- persistent activations ----
            lat_own = apool.tile([128, S], F16, name="lat_own", tag="lat_own")
            latT = [apool.tile([128, S], F16, name=f"latT{m}", tag=f"latT{m}")
                    for m in range(LAT // 128)]
            qT = [apool.tile([128, S], F16, name=f"qT{h}", tag=f"qT{h}")
                  for h in range(HEADS_PER_CORE)]
            kT = [apool.tile([128, S], F16, name=f"kT{h}", tag=f"kT{h}")
                  for h in range(HEADS_PER_CORE)]
            # v stored 4-heads-wide: col = 512*stt + 128*h + d
            vt_all = apool.tile([128, 4 * S], F16, name="vt_all", tag="vt_all")
            ctxT = [apool.tile([128, S], F16, name=f"ctxT{h}", tag=f"ctxT{h}")
                    for h in range(HEADS_PER_CORE)]

            # ================= phase 1: projections =================
            with tc.tile_pool(name="pproj", bufs=8, space="PSUM") as pproj:
                # PE warmup: HAM-warm the array while input DMAs stream in.
                warm = pproj.tile([128, 512], F32, name="warm", tag="pp")
                for _ in range(40):
                    nc.tensor.matmul(warm[:, 0:128], ones_t[:, 0:128],
                                     ones_t[:, 0:128], start=True, stop=True)

                def kmajor(groups, lhs_of, rhs_of, nk, out_of, copy_eng="scalar"):
                    """Accumulate len(groups) psum banks over nk chunks,
                    chunk-major so compute starts on the first DMA."""
                    pls = [pproj.tile([128, 512], F32, name=f"pp{i}", tag="pp")
                           for i in range(len(groups))]
                    for k in range(nk):
                        for i, g in enumerate(groups):
                            nc.tensor.matmul(pls[i][:], lhs_of(k, g), rhs_of(k, g),
                                             start=(k == 0), stop=(k == nk - 1))
                    for i, g in enumerate(groups):
                        eng = copy_eng(g) if callable(copy_eng) else copy_eng
                        if eng == "scalar":
                            nc.scalar.copy(out_of(g), pls[i][:])
                        else:
                            nc.vector.tensor_copy(out_of(g), pls[i][:])

                # sweep 1: lat_own (this core's 128 latent rows) + qT h=0,
                # interleaved chunk-major so the first sweep's compute rate
                # (~1.7us/chunk) matches the xt DMA stream rate
                kmajor(
                    [("lat", sb) for sb in range(NB)]
                    + [("q", sb) for sb in range(NB)],
                    lambda k, g: (wdkv[k][:, 0:128] if g[0] == "lat"
                                  else wq[k][:, 0:128]),
                    lambda k, g: xt[k][:, 512 * g[1]:512 * (g[1] + 1)],
                    KC,
                    lambda g: (lat_own if g[0] == "lat" else qT[0])[
                        :, 512 * g[1]:512 * (g[1] + 1)],
                    # lat copies on the (idle) DVE so the latent exchange
                    # uploads as early as possible; scalar is busy with its
                    # DMA-trigger backlog at this point
                    copy_eng=lambda g: "vector" if g[0] == "lat" else "scalar")

                # exchange latent halves with the paired core (rank order in
                # each group puts rows 0:128 first): DRAM bounce + AllGather
                with tc.tile_pool(name="ccdram", bufs=1, space="DRAM") as dpool:
                    cc_in = dpool.tile([128, S], F16, name="cc_in", tag="cc_in")
                    cc_out = dpool.tile([LAT, S], F16, name="cc_out",
                                        tag="cc_out")
                    nc.gpsimd.dma_start(cc_in[:], lat_own[:])
                    nc.gpsimd.collective_compute(
                        "AllGather",
                        mybir.AluOpType.bypass,
                        replica_groups=[[0, 2], [1, 3], [4, 6], [5, 7]],
                        ins=[cc_in.opt()],
                        outs=[cc_out.opt()],
                    )
                    nc.gpsimd.dma_start(latT[0][:], cc_out[0:128, :])
                    nc.gpsimd.dma_start(latT[1][:], cc_out[128:256, :])

                # sweeps 2-3: remaining qT heads (overlap the latent exchange)
                kmajor(
                    [(1 + dh, sb) for dh in range(2) for sb in range(NB)],
                    lambda k, g: wq[k][:, 128 * g[0]:128 * (g[0] + 1)],
                    lambda k, g: xt[k][:, 512 * g[1]:512 * (g[1] + 1)],
                    KC,
                    lambda g: qT[g[0]][:, 512 * g[1]:512 * (g[1] + 1)])
                kmajor(
                    [(3, sb) for sb in range(NB)],
                    lambda k, g: wq[k][:, 128 * g[0]:128 * (g[0] + 1)],
                    lambda k, g: xt[k][:, 512 * g[1]:512 * (g[1] + 1)],
                    KC,
                    lambda g: qT[g[0]][:, 512 * g[1]:512 * (g[1] + 1)])

                # kT_h = Wuk_h^T latT
                kmajor(
                    [(h, sb) for h in range(2) for sb in range(NB)],
                    lambda k, g: wuk[k][:, 128 * g[0]:128 * (g[0] + 1)],
                    lambda k, g: latT[k][:, 512 * g[1]:512 * (g[1] + 1)],
                    2,
                    lambda g: kT[g[0]][:, 512 * g[1]:512 * (g[1] + 1)],
                    copy_eng="vector")
                kmajor(
                    [(h, sb) for h in (2, 3) for sb in range(NB)],
                    lambda k, g: wuk[k][:, 128 * g[0]:128 * (g[0] + 1)],
                    lambda k, g: latT[k][:, 512 * g[1]:512 * (g[1] + 1)],
                    2,
                    lambda g: kT[g[0]][:, 512 * g[1]:512 * (g[1] + 1)],
                    copy_eng="vector")

                # v = latT^T Wuv, all 4 heads per matmul (512-wide moving
                # operand keeps LDWEIGHTS hidden); out tile = [s(128), 4h*hd]
                for stt in range(NT):
                    pv = pproj.tile([128, 512], F32, name="pv", tag="pp")
                    for m in range(LAT // 128):
                        nc.tensor.matmul(
                            pv[:],
                            latT[m][:, 128 * stt:128 * (stt + 1)],
                            wuv[m][:],
                            start=(m == 0), stop=(m == LAT // 128 - 1))
                    # alternate copy engines so the psum banks drain fast
                    # enough for the first attention scores
                    if stt % 2 == 0:
                        nc.vector.tensor_copy(
                            vt_all[:, 512 * stt:512 * (stt + 1)], pv[:])
                    else:
                        nc.scalar.copy(
                            vt_all[:, 512 * stt:512 * (stt + 1)], pv[:])

            # ========= phase 2: attention + interleaved out-proj =========
            # key tiles processed in pairs -> one [128,1024] exp per pair.
            # qb-outer / h-inner so each q-block's out-projection (PE-heavy,
            # ACT-idle) overlaps the next block's ACT-paced attention.
            with (
                tc.tile_pool(name="psc", bufs=2, space="PSUM") as psc,
                tc.tile_pool(name="pctx", bufs=2, space="PSUM") as pctx,
                tc.tile_pool(name="pden", bufs=2, space="PSUM") as pden,
            ):
                for qb in range(NB):
                    for h in range(HEADS_PER_CORE):
                        ps_ctx = pctx.tile([128, 512], F32, name="ps_ctx", tag="ctx")
                        ps_den = pden.tile([128, 512], F32, name="ps_den", tag="den")
                        nkt = 4 * qb + 4
                        exs_hold = None
                        den_started = False
                        for kt0 in range(0, nkt, 2):
                            pair = (kt0, kt0 + 1)
                            # valid q start (block-local) per kt; pair shares
                            # the wider (earlier) start col0 of ktA
                            djA = pair[0] - 4 * qb
                            col0 = 128 * djA if djA >= 0 else 0
                            qhi = 512 * (qb + 1)
                            ps_sc = psc.tile([128, 1024], F32, name="ps_sc",
                                             tag="sc")
                            ex = tpool.tile([128, 1024], F16, name="ex", tag="ex",
                                            bufs=3)
                            for half, kt in enumerate(pair):
                                dj = kt - 4 * qb
                                c = 128 * dj if dj >= 0 else 0
                                # fill with -huge ONLY the strip that both
                                # lies in the exp window [col0:1024] and is
                                # read by the pair-add/ctx consumers
                                # ([col0:512] and [512+col0:1024]); the rest
                                # of the invalid region is either below col0
                                # (never exp'd) or exp's garbage that no
                                # consumer reads.
                                flo = max(512 * half + col0, col0)
                                fhi = 512 * half + c
                                if fhi > flo:
                                    nc.tensor.matmul(
                                        ps_sc[:, flo:fhi],
                                        neg_t[:], ones_t[:, 0:fhi - flo],
                                        start=True, stop=True,
                                        skip_group_check=True)
                                nc.tensor.matmul(
                                    ps_sc[:, 512 * half + c:512 * (half + 1)],
                                    kT[h][:, 128 * kt:128 * (kt + 1)],
                                    qT[h][:, 512 * qb + c:qhi],
                                    start=True, stop=True,
                                    skip_group_check=True)
                            # one wide exp for the pair (psum -> sbuf fp16)
                            nc.scalar.activation(ex[:, col0:1024],
                                                 ps_sc[:, col0:1024], Exp,
                                                 scale=SCALE)
                            for half, kt in enumerate(pair):
                                dj = kt - 4 * qb
                                if dj >= 0:
                                    c = 128 * dj
                                    nc.vector.tensor_mul(
                                        ex[:, 512 * half + c:512 * half + c + 128],
                                        ex[:, 512 * half + c:512 * half + c + 128],
                                        mask_t[:])
                            # pair-sum on DVE halves the denominator matmuls;
                            # off-diagonal quads get a second-level quad-sum
                            # so one den matmul covers 4 key tiles
                            exs = tpool.tile([128, 512], F16, name="exs",
                                             tag="exs", bufs=3)
                            nc.vector.tensor_add(exs[:, col0:512],
                                                 ex[:, col0:512],
                                                 ex[:, 512 + col0:1024])
                            if (kt0 // 4) < qb:
                                if kt0 % 4 == 0:
                                    exs_hold = exs
                                else:
                                    exq = tpool.tile([128, 512], F16, name="exq",
                                                     tag="exq", bufs=2)
                                    nc.vector.tensor_add(exq[:], exs_hold[:],
                                                         exs[:])
                                    nc.tensor.matmul(
                                        ps_den[:], ones_t[:, 0:128], exq[:],
                                        start=(not den_started), stop=False)
                                    den_started = True
                            else:
                                nc.tensor.matmul(
                                    ps_den[:, col0:512],
                                    ones_t[:, 0:128],
                                    exs[:, col0:512],
                                    start=(not den_started),
                                    stop=(kt0 == nkt - 2))
                                den_started = True
                            for half, kt in enumerate(pair):
                                nc.tensor.matmul(
                                    ps_ctx[:, col0:512],
                                    vt_all[:, 512 * kt + 128 * h:
                                           512 * kt + 128 * (h + 1)],
                                    ex[:, 512 * half + col0:512 * (half + 1)],
                                    start=(kt0 == 0 and half == 0),
                                    stop=(kt == nkt - 1))
                        rden = tpool.tile([128, 512], F32, name="rden", tag="rden",
                                          bufs=2)
                        nc.vector.reciprocal_approx_fast(rden[:], ps_den[:])
                        nc.vector.tensor_mul(ctxT[h][:, 512 * qb:512 * (qb + 1)],
                                             ps_ctx[:], rden[:])

                    # out-projection for this q-block's 4 S-tiles
                    # (psum slots shared with the den tag). Copies alternate
                    # vector/scalar so the po psum slot frees at 2x rate;
                    # output staged+stored as fp16 (halves store volume),
                    # store triggers on sync/gpsimd (both idle here).
                    for stt in range(4 * qb, 4 * qb + 4):
                        for ob in range(NB):
                            po = pden.tile([128, 512], F32, name="po", tag="den")
                            for h in range(HEADS_PER_CORE):
                                nc.tensor.matmul(
                                    po[:],
                                    ctxT[h][:, 128 * stt:128 * (stt + 1)],
                                    wout[h][:, 512 * ob:512 * (ob + 1)],
                                    start=(h == 0), stop=(h == HEADS_PER_CORE - 1))
                            osb = tpool.tile([128, 512], F16, name="osb", tag="osb",
                                             bufs=4)
                            if (stt + ob) % 2 == 0:
                                nc.scalar.copy(osb[:], po[:])
                                st_eng = nc.gpsimd
                            else:
                                nc.vector.tensor_copy(osb[:], po[:])
                                st_eng = nc.sync
                            st_eng.dma_start(
                                out_d.ap()[128 * stt:128 * (stt + 1),
                                           512 * ob:512 * (ob + 1)],
                                osb[:])

    nc.compile()
    return nc


def _get_nc():
    if "nc" not in _CACHE:
        _CACHE["nc"] = _build()
    return _CACHE["nc"]


def _make_in_maps(x, W_query, W_DKV, W_UK, W_UV, W_out):
    mask = np.triu(np.ones((128, 128), dtype=np.float16))
    wdkv16 = W_DKV.astype(np.float16)
    xT16 = [x[b].T.astype(np.float16) for b in range(B)]
    in_maps = []
    for c in range(NCORES):
        b = c // 4
        g = c % 4
        mh = g // 2  # latent half this core computes (AllGather pairs 0-2, 1-3)
        cols = slice(512 * g, 512 * (g + 1))
        in_maps.append({
            "xt": xT16[b],
            "wq": W_query[:, cols].astype(np.float16),
            "wdkv": wdkv16[:, 128 * mh:128 * (mh + 1)],
            "wuk": W_UK[:, cols].astype(np.float16),
            "wuv": W_UV[:, cols].astype(np.float16),
            "wout": W_out[cols, :].astype(np.float16),
            "mask": mask,
        })
    return in_maps


def run_on_device(x, W_query, W_DKV, W_UK, W_UV, W_out, **run_kwargs):
    nc = _get_nc()
    in_maps = _make_in_maps(x, W_query, W_DKV, W_UK, W_UV, W_out)
    return bass_utils.run_bass_kernel_spmd(
        nc, in_maps, core_ids=list(range(NCORES)), **run_kwargs)


def kernel(x, W_query, W_DKV, W_UK, W_UV, W_out, b_out):
    x = np.asarray(x, dtype=np.float32)
    W_query = np.asarray(W_query, dtype=np.float32)
    W_DKV = np.asarray(W_DKV, dtype=np.float32)
    W_UK = np.asarray(W_UK, dtype=np.float32)
    W_UV = np.asarray(W_UV, dtype=np.float32)
    W_out = np.asarray(W_out, dtype=np.float32)
    b_out = np.asarray(b_out, dtype=np.float32)

    res = None
    for attempt in range(3):
        try:
            res = run_on_device(x, W_query, W_DKV, W_UK, W_UV, W_out)
            break
        except Exception:
            if attempt == 2:
                raise
    out = np.empty((B, S, DOUT), dtype=np.float32)
    for b in range(B):
        acc = res.results[4 * b]["out"].astype(np.float32)
        for g in range(1, 4):
            acc += res.results[4 * b + g]["out"].astype(np.float32)
        out[b] = acc + b_out[None, :]
    return out

